# revision 8
# baseline (speedup 1.0000x reference)
"""Trainium2 Bass kernel for nn_CustomizedSelfAttention.

Reference computation (per batch sample b):
    q = x @ Wq; k = x @ Wk; v = x @ Wv
    attn = softmax(q @ k.T * C**-0.5)          # [N, N]
    y = attn @ v @ Wp + bp + x                 # [N, C]
    pooled = mean(y, axis=0)                   # [C]
    out = relu(pooled @ Wf1 + bf1) @ Wf2 + bf2 # [C]

Key algebraic collapse (exact): only the token-mean of the attention output
is needed, so with  t[m] = sum_n softmax_row_n[m]  (column sums of the
attention matrix):
    mean_n(attn @ v) = (t/N) @ v = ((t/N) @ x) @ Wv
    pooled = ((t/N) @ x) @ (Wv @ Wp) + bp + mean_n(x)
This removes the O(N^2 C) attn@v matmul and the O(N C^2) v/p projections.
Also  q @ k.T * s = x @ (Wq @ Wk.T * s) @ x.T = x @ A @ x.T  with A fused on
the host, removing one more projection.

Per-core device pipeline (1 sample per core, weights replicated):
  phase 0 (per 1024-token chunk, overlapped with the x DMA stream):
    xbf[:, nt, :]  <- x rows, cast f32->bf16 on DMA (persistent all kernel)
    xT             <- PE 128x128 transposes, stored fp8 (feature-major)
    GT[d, n]       <- A.T-projection of the chunk, fp8 DoubleRow matmuls
  main loop (per pair of 128-row tiles):
    S   = GT.T xT  -> PSUM f32 (fp8 DoubleRow)
    E   = exp(S/ASC) -> e2[P, 2, N] fp8, row sums Z via ACT accum_out
    r   = (N/Z) in fp8 (DVE reduce + mul + reciprocal)
    tT  += per 128-col block: E_pair[:, blk].T @ r_pair  (fp8 DoubleRow,
           moving operand = r; the softmax normalization costs no DVE pass)
  tail (short, bf16 matmuls):
    TO  = [tT/N^2 | 1/N] bf16; Y2[2, C] = sum_j TO_j.T @ xbf_j  (u and xbar)
    pooled = u @ Wvp + bp + xbar; h = relu(pooled @ Wf1 + bf1)
    out = h @ Wf2 + bf2  (weights prefetched as bf16 at kernel start)
"""

import numpy as np
import ml_dtypes
from contextlib import ExitStack

import concourse.bass as bass
import concourse.tile as tile
from concourse import bacc, mybir
from concourse.bass_utils import run_bass_kernel_spmd

B, N, C = 8, 4096, 896
NCORES = 8
P = 128
CCH = C // P          # 7 feature chunks of 128
NT = N // P           # 32 token tiles of 128
MCH = 512             # S free-dim chunk (one PSUM bank)
NMC = N // MCH        # 8
TCH = 512             # token chunk for G projection
TCHM = 2 * TCH        # tokens per G chunk (fp8 DoubleRow)
NCH = N // TCHM       # 4
BF16 = mybir.dt.bfloat16
FP8 = mybir.dt.float8e4
F32 = mybir.dt.float32

_BF = ml_dtypes.bfloat16
_F8 = ml_dtypes.float8_e4m3

ASC = 128.0        # fp8 scale folded into A, undone in exp's affine
CG8 = 4            # fp8 c-groups of 256 (C padded 896 -> 1024)
DR = mybir.MatmulPerfMode.DoubleRow


def _build_body(ctx: ExitStack, tc: "tile.TileContext", aps: dict):
    nc = tc.nc
    x_d = aps["xc"]
    a_d = aps["abf"]
    bias_d = aps["biasR"]
    ident_d = aps["ident"]
    out_d = aps["outT"]

    const_pool = ctx.enter_context(tc.tile_pool(name="const", bufs=1))
    big_pool = ctx.enter_context(tc.tile_pool(name="big", bufs=1))
    e_pool = ctx.enter_context(tc.tile_pool(name="e", bufs=2))
    small_pool = ctx.enter_context(tc.tile_pool(name="small", bufs=4))
    tail_pool = ctx.enter_context(tc.tile_pool(name="tail", bufs=1))
    ps_pool = ctx.enter_context(tc.tile_pool(name="ps", bufs=3, space="PSUM"))
    acc_pool = ctx.enter_context(tc.tile_pool(name="acc", bufs=1, space="PSUM"))

    # --- constants / prefetches ---
    ident = const_pool.tile([P, P], BF16, tag="ident")
    nc.sync.dma_start(ident[:], ident_d)
    bias_sb = const_pool.tile([P, 3 * CCH], F32, tag="bias")
    nc.sync.dma_start(bias_sb[:], bias_d)
    # A pre-scaled by ASC, zero-padded to 1024 rows on host.
    a_sb = big_pool.tile([P, CG8, 2, C], FP8, tag="a")
    for bb in range(2 * CG8):
        nc.sync.dma_start(a_sb[:, bb // 2, bb % 2, :],
                          a_d[bb * P:(bb + 1) * P, :])
    # tail weights, cast to bf16 on DMA; queued behind the x loads so they
    # land long before the tail needs them
    w3 = big_pool.tile([P, 3, CCH, C], BF16, tag="w3")

    # --- persistent x representations ---
    xbf = big_pool.tile([P, NT, C], BF16, tag="xbf")
    xt = big_pool.tile([P, CG8, 2, N], FP8, tag="xt")
    gt = big_pool.tile([P, CG8, 2, N], FP8, tag="gt")
    nc.vector.memset(xt[:, CG8 - 1, 1, :], 0.0)   # pad features 896..1023
    nc.vector.memset(gt[:, CG8 - 1, 1, :], 0.0)

    # --- phase 0: stream x in, transpose, project G per 1024-token chunk ---
    for ch in range(NCH):
        # one cast-DMA per 4 token tiles: SWDGE descriptor-gen on Pool is
        # ~1us/DMA regardless of size, so batch the f32->bf16 x loads
        for j4 in range(0, TCHM // P, 4):
            nt0 = ch * (TCHM // P) + j4
            nc.gpsimd.dma_start(
                xbf[:, nt0:nt0 + 4, :],
                x_d[nt0 * P:(nt0 + 4) * P, :].rearrange(
                    "(j p) c -> p j c", p=P),
            )
        for q in range(TCHM // P):
            nt = ch * (TCHM // P) + q
            for cc in range(CCH):
                pt = ps_pool.tile([P, P], BF16, tag="ps", name="pt")
                nc.tensor.transpose(pt[:], xbf[:, nt, cc * P:(cc + 1) * P],
                                    ident[:])
                dst = xt[:, cc // 2, cc % 2, nt * P:(nt + 1) * P]
                if cc % 2 == 0:
                    nc.vector.tensor_copy(dst, pt[:])
                else:
                    nc.scalar.copy(dst, pt[:])
        for dd in range(CCH):
            gp = ps_pool.tile([P, 2, TCH], F32, tag="ps", name="gp")
            for g in range(CG8):
                for h2 in range(2):
                    nc.tensor.matmul(
                        gp[:, h2, :], a_sb[:, g, :, dd * P:(dd + 1) * P],
                        xt[:, g, :,
                           ch * TCHM + h2 * TCH:ch * TCHM + (h2 + 1) * TCH],
                        start=(g == 0), stop=(g == CG8 - 1),
                        perf_mode=DR, skip_group_check=True,
                    )
            dst = gt[:, dd // 2, dd % 2, ch * TCHM:(ch + 1) * TCHM]
            if dd % 2 == 0:
                nc.vector.tensor_copy(dst, gp[:])
            else:
                nc.scalar.copy(dst, gp[:])
    for i in range(3):
        wk = ("wvp", "wf1", "wf2")[i]
        nc.gpsimd.dma_start(w3[:, i, :, :],
                            aps[wk].rearrange("(cc p) e -> p cc e", p=P))

    # --- main loop: S -> exp -> weighted colsum, per pair of row tiles ---
    NPAIR = NT // 2
    tT = acc_pool.tile([P, NT], F32, tag="acc")  # t * N accumulator
    pend = []  # deferred colsum emission: (pair, e2, rf2)

    def emit_colsum(pair, e2, rf2):
        for j in range(NT):
            nc.tensor.matmul(
                tT[:, j:j + 1], e2[:, :, j * P:(j + 1) * P], rf2[:],
                start=(pair == 0 and j == 0),
                stop=(pair == NPAIR - 1 and j == NT - 1),
                perf_mode=DR, skip_group_check=True,
            )

    for pair in range(NPAIR):
        e2 = e_pool.tile([P, 2, N], FP8, tag="e2")
        rf2 = small_pool.tile([P, 2, 1], FP8, tag="rf2")
        for h in range(2):
            nt = 2 * pair + h
            zp = small_pool.tile([P, 4], F32, tag="zp")
            for ti, (mj0, nck) in enumerate(((0, 2), (2, 2), (4, 2), (6, 2))):
                sps = ps_pool.tile([P, nck, MCH], F32, tag="ps", name="sps")
                for g in range(CG8):
                    for k3 in range(nck):
                        mj = mj0 + k3
                        nc.tensor.matmul(
                            sps[:, k3, :], gt[:, g, :, nt * P:(nt + 1) * P],
                            xt[:, g, :, mj * MCH:(mj + 1) * MCH],
                            start=(g == 0), stop=(g == CG8 - 1),
                            perf_mode=DR, skip_group_check=True,
                        )
                nc.scalar.activation(
                    e2[:, h, mj0 * MCH:(mj0 + nck) * MCH], sps[:],
                    mybir.ActivationFunctionType.Exp,
                    scale=1.0 / ASC,
                    accum_out=zp[:, ti:ti + 1],
                )
            # PE keeps streaming the next tile's S while this runs on DVE
            if h == 0 and pend:
                emit_colsum(*pend.pop(0))
            z = small_pool.tile([P, 1], F32, tag="z")
            nc.vector.reduce_sum(z[:], zp[:], axis=mybir.AxisListType.X)
            zn = small_pool.tile([P, 1], F32, tag="zn")
            nc.vector.tensor_scalar_mul(zn[:], z[:], 1.0 / N)
            # r = N/Z ~ O(1); fp8 quantization of r is ~6% per row but the
            # colsum averages 4096 independent rows -> ~0.1% on t
            with nc.allow_low_precision("r stat-averages over 4096 rows"):
                nc.vector.reciprocal(rf2[:, h, :], zn[:])
        pend.append((pair, e2, rf2))
    while pend:
        emit_colsum(*pend.pop(0))

    # --- tail ---
    # TO[:, j, 0] = (t*N)[j-chunk] / N^2 = t/N ; TO[:, j, 1] = 1/N
    TO = tail_pool.tile([P, NT, 2], BF16, tag="to")
    nc.vector.memset(TO[:, :, 1], 1.0 / N)
    nc.scalar.mul(TO[:, :, 0], tT[:], 1.0 / (N * N))

    # Y2[0, c] = u = (t/N) @ x ; Y2[1, c] = xbar (mean over tokens)
    Y2 = ps_pool.tile([2, C], F32, tag="ps", name="y2")
    for j in range(NT):
        for (o, w) in ((0, 512), (512, 384)):
            nc.tensor.matmul(
                Y2[:, o:o + w], TO[:, j, :], xbf[:, j, o:o + w],
                start=(j == 0), stop=(j == NT - 1),
                skip_group_check=True,
            )
    y2S = tail_pool.tile([2, C], F32, tag="y2s")
    nc.scalar.copy(y2S[:], Y2[:])
    identF2 = tail_pool.tile([2, 2], F32, tag="idf2")
    nc.vector.tensor_copy(identF2[:], ident[0:2, 0:2])
    uxS = tail_pool.tile([P, CCH, 2], F32, tag="ux")
    for cc in range(CCH):
        ptx = ps_pool.tile([P, 2], F32, tag="ps", name="ptx")
        nc.tensor.transpose(ptx[:], y2S[:, cc * P:(cc + 1) * P], identF2[:])
        nc.vector.tensor_copy(uxS[:, cc, :], ptx[:])

    def matvec(wi, vec_cols, out_psum):
        for ee in range(CCH):
            for cc in range(CCH):
                nc.tensor.matmul(
                    out_psum[:, ee:ee + 1],
                    w3[:, wi, cc, ee * P:(ee + 1) * P],
                    vec_cols(cc),
                    start=(cc == 0), stop=(cc == CCH - 1),
                )

    uxB = tail_pool.tile([P, CCH, 1], BF16, tag="uxb")
    nc.vector.tensor_copy(uxB[:], uxS[:, :, 0:1])
    P2 = ps_pool.tile([P, CCH], F32, tag="ps", name="p2")
    matvec(0, lambda cc: uxB[:, cc, :], P2)
    pooledS = tail_pool.tile([P, CCH], F32, tag="pooled")
    nc.vector.tensor_add(pooledS[:], P2[:], uxS[:, :, 1])
    nc.vector.tensor_add(pooledS[:], pooledS[:], bias_sb[:, 0:CCH])
    pooledB = tail_pool.tile([P, CCH], BF16, tag="pooledb")
    nc.vector.tensor_copy(pooledB[:], pooledS[:])

    H2 = ps_pool.tile([P, CCH], F32, tag="ps", name="h2")
    matvec(1, lambda cc: pooledB[:, cc:cc + 1], H2)
    hS = tail_pool.tile([P, CCH], F32, tag="h")
    nc.vector.tensor_add(hS[:], H2[:], bias_sb[:, CCH:2 * CCH])
    nc.vector.tensor_scalar_max(hS[:], hS[:], 0.0)
    hB = tail_pool.tile([P, CCH], BF16, tag="hb")
    nc.vector.tensor_copy(hB[:], hS[:])

    O2 = ps_pool.tile([P, CCH], F32, tag="ps", name="o2")
    matvec(2, lambda cc: hB[:, cc:cc + 1], O2)
    outS = tail_pool.tile([P, CCH], F32, tag="out")
    nc.vector.tensor_add(outS[:], O2[:], bias_sb[:, 2 * CCH:3 * CCH])
    nc.sync.dma_start(out_d, outS[:])


_NC_CACHE = {}


def build_nc(reps=1):
    key = ("nc", reps)
    if key in _NC_CACHE:
        return _NC_CACHE[key]
    nc = bacc.Bacc(
        "TRN2", target_bir_lowering=False, debug=False,
        enable_asserts=False, num_devices=NCORES,
    )
    aps = {
        "xc": nc.dram_tensor("xc", [N, C], F32, kind="ExternalInput").ap(),
        "abf": nc.dram_tensor("abf", [2 * CG8 * P, C], FP8,
                              kind="ExternalInput").ap(),
        "wvp": nc.dram_tensor("wvp", [C, C], F32, kind="ExternalInput").ap(),
        "wf1": nc.dram_tensor("wf1", [C, C], F32, kind="ExternalInput").ap(),
        "wf2": nc.dram_tensor("wf2", [C, C], F32, kind="ExternalInput").ap(),
        "biasR": nc.dram_tensor("biasR", [P, 3 * CCH], F32,
                                kind="ExternalInput").ap(),
        "ident": nc.dram_tensor("ident", [P, P], BF16,
                                kind="ExternalInput").ap(),
        "outT": nc.dram_tensor("outT", [P, CCH], F32,
                               kind="ExternalOutput").ap(),
    }
    with tile.TileContext(nc) as tc:
        for _ in range(reps):
            with ExitStack() as ctx:
                _build_body(ctx, tc, aps)
    nc.compile()
    _NC_CACHE[key] = nc
    return nc


def prep_in_maps(x_, Wq, Wk, Wv, Wp, bp, Wf1, bf1, Wf2, bf2):
    f32 = np.float32
    x_ = np.ascontiguousarray(np.asarray(x_, dtype=f32))
    A = (np.asarray(Wq, f32) @ np.asarray(Wk, f32).T) * np.float32(C ** -0.5)
    a_pad = np.zeros((2 * CG8 * P, C), f32)
    a_pad[:C] = A * np.float32(ASC)
    abf = np.ascontiguousarray(a_pad.astype(_F8))
    wvp = np.ascontiguousarray(np.asarray(Wv, f32) @ np.asarray(Wp, f32))
    wf1 = np.ascontiguousarray(np.asarray(Wf1, f32))
    wf2 = np.ascontiguousarray(np.asarray(Wf2, f32))
    biasR = np.concatenate(
        [np.asarray(b, f32).reshape(CCH, P).T for b in (bp, bf1, bf2)], axis=1
    )
    biasR = np.ascontiguousarray(biasR)
    ident = np.eye(P, dtype=_BF)
    shared = {
        "abf": abf, "wvp": wvp, "wf1": wf1, "wf2": wf2,
        "biasR": biasR, "ident": ident,
    }
    return [dict(shared, xc=np.ascontiguousarray(x_[b])) for b in range(B)]


def assemble_output(results):
    out = np.empty((B, C), dtype=np.float32)
    for b in range(B):
        out[b] = np.asarray(results[b]["outT"], np.float32).T.reshape(C)
    return out


def kernel(**inputs) -> np.ndarray:
    nc = build_nc()
    in_maps = prep_in_maps(**inputs)
    res = run_bass_kernel_spmd(nc, in_maps, list(range(NCORES)))
    return assemble_output(res.results)


if __name__ == "__main__":
    import reference as R
    inp = {k: np.asarray(v) for k, v in R.setup_inputs().items()}
    out = kernel(**inp)
    print(out.shape, out.dtype)


# revision 9
# speedup vs baseline: 1.0725x; 1.0725x over previous
"""Trainium2 Bass kernel for nn_CustomizedSelfAttention.

Reference computation (per batch sample b):
    q = x @ Wq; k = x @ Wk; v = x @ Wv
    attn = softmax(q @ k.T * C**-0.5)          # [N, N]
    y = attn @ v @ Wp + bp + x                 # [N, C]
    pooled = mean(y, axis=0)                   # [C]
    out = relu(pooled @ Wf1 + bf1) @ Wf2 + bf2 # [C]

Key algebraic collapse (exact): only the token-mean of the attention output
is needed, so with  t[m] = sum_n softmax_row_n[m]  (column sums of the
attention matrix):
    mean_n(attn @ v) = (t/N) @ v = ((t/N) @ x) @ Wv
    pooled = ((t/N) @ x) @ (Wv @ Wp) + bp + mean_n(x)
This removes the O(N^2 C) attn@v matmul and the O(N C^2) v/p projections.
Also  q @ k.T * s = x @ (Wq @ Wk.T * s) @ x.T = x @ A @ x.T  with A fused on
the host, removing one more projection.

Per-core device pipeline (1 sample per core, weights replicated):
  phase 0 (per 1024-token chunk, overlapped with the x DMA stream):
    xbf[:, nt, :]  <- x rows, cast f32->bf16 on DMA (persistent all kernel)
    xT             <- PE 128x128 transposes, stored fp8 (feature-major)
    GT[d, n]       <- A.T-projection of the chunk, fp8 DoubleRow matmuls
  main loop (per pair of 128-row tiles):
    S   = GT.T xT  -> PSUM f32 (fp8 DoubleRow)
    E   = exp(S/ASC) -> e2[P, 2, N] fp8, row sums Z via ACT accum_out
    r   = (N/Z) in fp8 (DVE reduce + mul + reciprocal)
    tT  += per 128-col block: E_pair[:, blk].T @ r_pair  (fp8 DoubleRow,
           moving operand = r; the softmax normalization costs no DVE pass)
  tail (short, bf16 matmuls):
    TO  = [tT/N^2 | 1/N] bf16; Y2[2, C] = sum_j TO_j.T @ xbf_j  (u and xbar)
    pooled = u @ Wvp + bp + xbar; h = relu(pooled @ Wf1 + bf1)
    out = h @ Wf2 + bf2  (weights prefetched as bf16 at kernel start)
"""

import numpy as np
import ml_dtypes
from contextlib import ExitStack

import concourse.bass as bass
import concourse.tile as tile
from concourse import bacc, mybir
from concourse.bass_utils import run_bass_kernel_spmd

B, N, C = 8, 4096, 896
NCORES = 8
P = 128
CCH = C // P          # 7 feature chunks of 128
NT = N // P           # 32 token tiles of 128
MCH = 512             # S free-dim chunk (one PSUM bank)
NMC = N // MCH        # 8
TCH = 512             # token chunk for G projection
TCHM = 2 * TCH        # tokens per G chunk (fp8 DoubleRow)
NCH = N // TCHM       # 4
BF16 = mybir.dt.bfloat16
FP8 = mybir.dt.float8e4
F32 = mybir.dt.float32

_BF = ml_dtypes.bfloat16
_F8 = ml_dtypes.float8_e4m3

ASC = 128.0        # fp8 scale folded into A, undone in exp's affine
CG8 = 4            # fp8 c-groups of 256 (C padded 896 -> 1024)
DR = mybir.MatmulPerfMode.DoubleRow


def _build_body(ctx: ExitStack, tc: "tile.TileContext", aps: dict):
    nc = tc.nc
    x_d = aps["xc"]
    a_d = aps["abf"]
    bias_d = aps["biasR"]
    ident_d = aps["ident"]
    out_d = aps["outT"]

    const_pool = ctx.enter_context(tc.tile_pool(name="const", bufs=1))
    big_pool = ctx.enter_context(tc.tile_pool(name="big", bufs=1))
    e_pool = ctx.enter_context(tc.tile_pool(name="e", bufs=2))
    small_pool = ctx.enter_context(tc.tile_pool(name="small", bufs=4))
    tail_pool = ctx.enter_context(tc.tile_pool(name="tail", bufs=1))
    ps_pool = ctx.enter_context(tc.tile_pool(name="ps", bufs=3, space="PSUM"))
    acc_pool = ctx.enter_context(tc.tile_pool(name="acc", bufs=1, space="PSUM"))

    # --- constants / prefetches ---
    ident = const_pool.tile([P, P], BF16, tag="ident")
    nc.sync.dma_start(ident[:], ident_d)
    bias_sb = const_pool.tile([P, 3 * CCH], F32, tag="bias")
    nc.sync.dma_start(bias_sb[:], bias_d)
    # A pre-scaled by ASC, zero-padded to 1024 rows on host.
    a_sb = big_pool.tile([P, CG8, 2, C], FP8, tag="a")
    for bb in range(2 * CG8):
        nc.sync.dma_start(a_sb[:, bb // 2, bb % 2, :],
                          a_d[bb * P:(bb + 1) * P, :])
    # tail weights, cast to bf16 on DMA; queued behind the x loads so they
    # land long before the tail needs them
    w3 = big_pool.tile([P, 3, CCH, C], BF16, tag="w3")

    # --- persistent x representations ---
    xbf = big_pool.tile([P, NT, C], BF16, tag="xbf")
    xt = big_pool.tile([P, CG8, 2, N], FP8, tag="xt")
    gt = big_pool.tile([P, CG8, 2, N], FP8, tag="gt")
    nc.vector.memset(xt[:, CG8 - 1, 1, :], 0.0)   # pad features 896..1023
    nc.vector.memset(gt[:, CG8 - 1, 1, :], 0.0)

    # --- phase 0: stream x in, transpose, project G per 1024-token chunk ---
    for ch in range(NCH):
        # one cast-DMA per 4 token tiles: SWDGE descriptor-gen on Pool is
        # ~1us/DMA regardless of size, so batch the f32->bf16 x loads
        for j4 in range(0, TCHM // P, 4):
            nt0 = ch * (TCHM // P) + j4
            nc.gpsimd.dma_start(
                xbf[:, nt0:nt0 + 4, :],
                x_d[nt0 * P:(nt0 + 4) * P, :].rearrange(
                    "(j p) c -> p j c", p=P),
            )
        for q in range(TCHM // P):
            nt = ch * (TCHM // P) + q
            for cp in range(4):
                cc = 2 * cp
                if cp < 3:
                    pt = ps_pool.tile([P, 2, P], BF16, tag="ps", name="pt")
                    for k in range(2):
                        nc.tensor.transpose(
                            pt[:, k, :],
                            xbf[:, nt, (cc + k) * P:(cc + k + 1) * P],
                            ident[:])
                    dst = xt[:, cp, :, nt * P:(nt + 1) * P]
                else:
                    pt = ps_pool.tile([P, P], BF16, tag="ps", name="pt1")
                    nc.tensor.transpose(
                        pt[:], xbf[:, nt, cc * P:(cc + 1) * P], ident[:])
                    dst = xt[:, cp, 0, nt * P:(nt + 1) * P]
                if (nt * 4 + cp) % 2 == 0:
                    nc.vector.tensor_copy(dst, pt[:])
                else:
                    nc.scalar.copy(dst, pt[:])
        for dd in range(CCH):
            gp = ps_pool.tile([P, 2, TCH], F32, tag="ps", name="gp")
            for g in range(CG8):
                for h2 in range(2):
                    nc.tensor.matmul(
                        gp[:, h2, :], a_sb[:, g, :, dd * P:(dd + 1) * P],
                        xt[:, g, :,
                           ch * TCHM + h2 * TCH:ch * TCHM + (h2 + 1) * TCH],
                        start=(g == 0), stop=(g == CG8 - 1),
                        perf_mode=DR, skip_group_check=True,
                    )
            dst = gt[:, dd // 2, dd % 2, ch * TCHM:(ch + 1) * TCHM]
            if dd % 2 == 0:
                nc.vector.tensor_copy(dst, gp[:])
            else:
                nc.scalar.copy(dst, gp[:])
    for i in range(3):
        wk = ("wvp", "wf1", "wf2")[i]
        nc.gpsimd.dma_start(w3[:, i, :, :],
                            aps[wk].rearrange("(cc p) e -> p cc e", p=P))

    # --- main loop: S -> exp -> weighted colsum, per pair of row tiles ---
    NPAIR = NT // 2
    tT = acc_pool.tile([P, NT], F32, tag="acc")  # t * N accumulator
    pend = []  # deferred colsum emission: (pair, e2, rf2)

    def emit_colsum(pair, e2, rf2):
        for j in range(NT):
            nc.tensor.matmul(
                tT[:, j:j + 1], e2[:, :, j * P:(j + 1) * P], rf2[:],
                start=(pair == 0 and j == 0),
                stop=(pair == NPAIR - 1 and j == NT - 1),
                perf_mode=DR, skip_group_check=True,
            )

    for pair in range(NPAIR):
        e2 = e_pool.tile([P, 2, N], FP8, tag="e2")
        rf2 = small_pool.tile([P, 2, 1], FP8, tag="rf2")
        for h in range(2):
            nt = 2 * pair + h
            zp = small_pool.tile([P, 4], F32, tag="zp")
            for ti, (mj0, nck) in enumerate(((0, 2), (2, 2), (4, 2), (6, 2))):
                sps = ps_pool.tile([P, nck, MCH], F32, tag="ps", name="sps")
                for g in range(CG8):
                    for k3 in range(nck):
                        mj = mj0 + k3
                        nc.tensor.matmul(
                            sps[:, k3, :], gt[:, g, :, nt * P:(nt + 1) * P],
                            xt[:, g, :, mj * MCH:(mj + 1) * MCH],
                            start=(g == 0), stop=(g == CG8 - 1),
                            perf_mode=DR, skip_group_check=True,
                        )
                nc.scalar.activation(
                    e2[:, h, mj0 * MCH:(mj0 + nck) * MCH], sps[:],
                    mybir.ActivationFunctionType.Exp,
                    scale=1.0 / ASC,
                    accum_out=zp[:, ti:ti + 1],
                )
            # PE keeps streaming the next tile's S while this runs on DVE
            if h == 0 and pend:
                emit_colsum(*pend.pop(0))
            z = small_pool.tile([P, 1], F32, tag="z")
            nc.vector.reduce_sum(z[:], zp[:], axis=mybir.AxisListType.X)
            zn = small_pool.tile([P, 1], F32, tag="zn")
            nc.vector.tensor_scalar_mul(zn[:], z[:], 1.0 / N)
            # r = N/Z ~ O(1); fp8 quantization of r is ~6% per row but the
            # colsum averages 4096 independent rows -> ~0.1% on t
            with nc.allow_low_precision("r stat-averages over 4096 rows"):
                nc.vector.reciprocal(rf2[:, h, :], zn[:])
        pend.append((pair, e2, rf2))
    while pend:
        emit_colsum(*pend.pop(0))

    # --- tail ---
    # TO[:, j, 0] = (t*N)[j-chunk] / N^2 = t/N ; TO[:, j, 1] = 1/N
    TO = tail_pool.tile([P, NT, 2], BF16, tag="to")
    nc.vector.memset(TO[:, :, 1], 1.0 / N)
    nc.scalar.mul(TO[:, :, 0], tT[:], 1.0 / (N * N))

    # Y2[0, c] = u = (t/N) @ x ; Y2[1, c] = xbar (mean over tokens)
    Y2 = ps_pool.tile([2, C], F32, tag="ps", name="y2")
    for j in range(NT):
        for (o, w) in ((0, 512), (512, 384)):
            nc.tensor.matmul(
                Y2[:, o:o + w], TO[:, j, :], xbf[:, j, o:o + w],
                start=(j == 0), stop=(j == NT - 1),
                skip_group_check=True,
            )
    y2S = tail_pool.tile([2, C], F32, tag="y2s")
    nc.scalar.copy(y2S[:], Y2[:])
    identF2 = tail_pool.tile([2, 2], F32, tag="idf2")
    nc.vector.tensor_copy(identF2[:], ident[0:2, 0:2])
    uxS = tail_pool.tile([P, CCH, 2], F32, tag="ux")
    for cc in range(CCH):
        ptx = ps_pool.tile([P, 2], F32, tag="ps", name="ptx")
        nc.tensor.transpose(ptx[:], y2S[:, cc * P:(cc + 1) * P], identF2[:])
        nc.vector.tensor_copy(uxS[:, cc, :], ptx[:])

    def matvec(wi, vec_cols, out_psum):
        for ee in range(CCH):
            for cc in range(CCH):
                nc.tensor.matmul(
                    out_psum[:, ee:ee + 1],
                    w3[:, wi, cc, ee * P:(ee + 1) * P],
                    vec_cols(cc),
                    start=(cc == 0), stop=(cc == CCH - 1),
                )

    uxB = tail_pool.tile([P, CCH, 1], BF16, tag="uxb")
    nc.vector.tensor_copy(uxB[:], uxS[:, :, 0:1])
    P2 = ps_pool.tile([P, CCH], F32, tag="ps", name="p2")
    matvec(0, lambda cc: uxB[:, cc, :], P2)
    pooledS = tail_pool.tile([P, CCH], F32, tag="pooled")
    nc.vector.tensor_add(pooledS[:], P2[:], uxS[:, :, 1])
    nc.vector.tensor_add(pooledS[:], pooledS[:], bias_sb[:, 0:CCH])
    pooledB = tail_pool.tile([P, CCH], BF16, tag="pooledb")
    nc.vector.tensor_copy(pooledB[:], pooledS[:])

    H2 = ps_pool.tile([P, CCH], F32, tag="ps", name="h2")
    matvec(1, lambda cc: pooledB[:, cc:cc + 1], H2)
    hS = tail_pool.tile([P, CCH], F32, tag="h")
    nc.vector.tensor_add(hS[:], H2[:], bias_sb[:, CCH:2 * CCH])
    nc.vector.tensor_scalar_max(hS[:], hS[:], 0.0)
    hB = tail_pool.tile([P, CCH], BF16, tag="hb")
    nc.vector.tensor_copy(hB[:], hS[:])

    O2 = ps_pool.tile([P, CCH], F32, tag="ps", name="o2")
    matvec(2, lambda cc: hB[:, cc:cc + 1], O2)
    outS = tail_pool.tile([P, CCH], F32, tag="out")
    nc.vector.tensor_add(outS[:], O2[:], bias_sb[:, 2 * CCH:3 * CCH])
    nc.sync.dma_start(out_d, outS[:])


_NC_CACHE = {}


def build_nc(reps=1):
    key = ("nc", reps)
    if key in _NC_CACHE:
        return _NC_CACHE[key]
    nc = bacc.Bacc(
        "TRN2", target_bir_lowering=False, debug=False,
        enable_asserts=False, num_devices=NCORES,
    )
    aps = {
        "xc": nc.dram_tensor("xc", [N, C], F32, kind="ExternalInput").ap(),
        "abf": nc.dram_tensor("abf", [2 * CG8 * P, C], FP8,
                              kind="ExternalInput").ap(),
        "wvp": nc.dram_tensor("wvp", [C, C], F32, kind="ExternalInput").ap(),
        "wf1": nc.dram_tensor("wf1", [C, C], F32, kind="ExternalInput").ap(),
        "wf2": nc.dram_tensor("wf2", [C, C], F32, kind="ExternalInput").ap(),
        "biasR": nc.dram_tensor("biasR", [P, 3 * CCH], F32,
                                kind="ExternalInput").ap(),
        "ident": nc.dram_tensor("ident", [P, P], BF16,
                                kind="ExternalInput").ap(),
        "outT": nc.dram_tensor("outT", [P, CCH], F32,
                               kind="ExternalOutput").ap(),
    }
    with tile.TileContext(nc) as tc:
        for _ in range(reps):
            with ExitStack() as ctx:
                _build_body(ctx, tc, aps)
    nc.compile()
    _NC_CACHE[key] = nc
    return nc


def prep_in_maps(x_, Wq, Wk, Wv, Wp, bp, Wf1, bf1, Wf2, bf2):
    f32 = np.float32
    x_ = np.ascontiguousarray(np.asarray(x_, dtype=f32))
    A = (np.asarray(Wq, f32) @ np.asarray(Wk, f32).T) * np.float32(C ** -0.5)
    a_pad = np.zeros((2 * CG8 * P, C), f32)
    a_pad[:C] = A * np.float32(ASC)
    abf = np.ascontiguousarray(a_pad.astype(_F8))
    wvp = np.ascontiguousarray(np.asarray(Wv, f32) @ np.asarray(Wp, f32))
    wf1 = np.ascontiguousarray(np.asarray(Wf1, f32))
    wf2 = np.ascontiguousarray(np.asarray(Wf2, f32))
    biasR = np.concatenate(
        [np.asarray(b, f32).reshape(CCH, P).T for b in (bp, bf1, bf2)], axis=1
    )
    biasR = np.ascontiguousarray(biasR)
    ident = np.eye(P, dtype=_BF)
    shared = {
        "abf": abf, "wvp": wvp, "wf1": wf1, "wf2": wf2,
        "biasR": biasR, "ident": ident,
    }
    return [dict(shared, xc=np.ascontiguousarray(x_[b])) for b in range(B)]


def assemble_output(results):
    out = np.empty((B, C), dtype=np.float32)
    for b in range(B):
        out[b] = np.asarray(results[b]["outT"], np.float32).T.reshape(C)
    return out


def kernel(**inputs) -> np.ndarray:
    nc = build_nc()
    in_maps = prep_in_maps(**inputs)
    res = run_bass_kernel_spmd(nc, in_maps, list(range(NCORES)))
    return assemble_output(res.results)


if __name__ == "__main__":
    import reference as R
    inp = {k: np.asarray(v) for k, v in R.setup_inputs().items()}
    out = kernel(**inp)
    print(out.shape, out.dtype)
